# revision 9
# baseline (speedup 1.0000x reference)
import sys, os
sys.path.insert(0, "/opt/trn_rl_repo")
import numpy as np
import ml_dtypes

BF16 = ml_dtypes.bfloat16
SR = 8
MOFF = 17
NOFF = 289          # 17*17
MID = 144
OUT = 49
H = W = 128
BS = 2
NBLK = 4            # row blocks per batch
BR = 32             # output rows per block
MR = BR + 10        # mv rows per core (halo 5 each side) = 42
F2R = BR + 26       # f2 rows per core (halo 13 each side) = 58
W2 = W + 16         # f2 padded width = 144

_cache = {}


def _build():
    import concourse.bacc as bacc
    import concourse.bass as bass
    import concourse.mybir as mybir
    import concourse.tile as tile

    f32 = mybir.dt.float32
    bf16 = mybir.dt.bfloat16
    ALU = mybir.AluOpType
    ACT = mybir.ActivationFunctionType

    nc = bacc.Bacc(None, target_bir_lowering=False)
    f1d = nc.dram_tensor("f1s", [128, MR, W], bf16, kind="ExternalInput")
    f2d = nc.dram_tensor("f2s", [128, F2R, W2], bf16, kind="ExternalInput")
    onesd = nc.dram_tensor("onescols", [128, 64, 64], bf16, kind="ExternalInput")
    diagd = nc.dram_tensor("diag_aw", [128, 3, 49, 128], bf16, kind="ExternalInput")
    attbd = nc.dram_tensor("attb", [128, 3], f32, kind="ExternalInput")
    c1d = nc.dram_tensor("c1t", [128, 3, 9, MID], bf16, kind="ExternalInput")
    k1d = nc.dram_tensor("k1", [128, 2], f32, kind="ExternalInput")
    b1d = nc.dram_tensor("b1p", [128, 2], f32, kind="ExternalInput")
    c2d = nc.dram_tensor("c2t", [128, 2, 9, OUT], bf16, kind="ExternalInput")
    k2d = nc.dram_tensor("k2", [128, 1], f32, kind="ExternalInput")
    b2d = nc.dram_tensor("b2p", [128, 1], f32, kind="ExternalInput")
    xmd = nc.dram_tensor("xmask", [128, 36], bf16, kind="ExternalInput")
    outd = nc.dram_tensor("y", [OUT, BR, W], f32, kind="ExternalOutput")

    # cv pixel chunks over the 42 mv rows
    cvch = [(t0, min(4, MR - t0)) for t0 in range(0, MR, 4)]
    ogrp = [(g * 64, min(64, NOFF - g * 64)) for g in range(5)]

    with tile.TileContext(nc) as tc:
        with (
            tc.tile_pool(name="persist", bufs=1) as P,
            tc.tile_pool(name="prod", bufs=3) as PR,
            tc.tile_pool(name="psum", bufs=2, space="PSUM") as PS,
            tc.tile_pool(name="tmp", bufs=2) as T,
            tc.tile_pool(name="srep", bufs=2) as SRP,
            tc.tile_pool(name="dram", bufs=1, space="DRAM") as DR,
        ):
            f1t = P.tile([128, MR, W], bf16)
            f2t = P.tile([128, F2R, W2], bf16)
            ones = P.tile([128, 64, 64], bf16)
            diag = P.tile([128, 3, 49, 128], bf16)
            attb = P.tile([128, 3], f32)
            c1t = P.tile([128, 3, 9, MID], bf16)
            k1 = P.tile([128, 2], f32)
            b1 = P.tile([128, 2], f32)
            c2t = P.tile([128, 2, 9, OUT], bf16)
            k2 = P.tile([128, 1], f32)
            b2 = P.tile([128, 1], f32)
            xm = P.tile([128, 36], bf16)
            for dst, src in [(f1t, f1d), (f2t, f2d), (ones, onesd), (diag, diagd),
                             (attb, attbd), (c1t, c1d), (k1, k1d), (b1, b1d),
                             (c2t, c2d), (k2, k2d), (b2, b2d), (xm, xmd)]:
                nc.sync.dma_start(dst[:], src[:])

            # ---- s = 1/(128*max(||f1||,1e-12)) per pixel -------------------
            sdr = DR.tile([1, MR * W], f32)
            for t0, rc in cvch:
                fsq = PR.tile([128, 4, W], bf16, tag="prod")
                nc.vector.tensor_mul(fsq[:, 0:rc, :], f1t[:, t0:t0 + rc, :],
                                     f1t[:, t0:t0 + rc, :])
                ps = PS.tile([64, 512], f32, tag="cvps")
                nc.tensor.matmul(ps[0:1, 0:rc * W],
                                 ones[:, 0, 0:1],
                                 fsq[:, 0:rc, :].rearrange("c a b -> c (a b)"),
                                 start=True, stop=True)
                tt = T.tile([1, 512], f32, tag="stmp")
                nc.vector.tensor_scalar(tt[0:1, 0:rc * W], ps[0:1, 0:rc * W],
                                        16384.0, 1.6384e-20, op0=ALU.mult, op1=ALU.max)
                nc.scalar.activation(tt[0:1, 0:rc * W], tt[0:1, 0:rc * W], ACT.Sqrt)
                st = T.tile([1, 512], f32, tag="srow")
                nc.vector.reciprocal(st[0:1, 0:rc * W], tt[0:1, 0:rc * W])
                nc.sync.dma_start(sdr[0:1, t0 * W:(t0 + rc) * W], st[0:1, 0:rc * W])

            # ---- cost volume -> mv (channel layout, padded) ----------------
            mv = [P.tile([128, 48, 134], bf16, tag=f"mv{i}", name=f"mv{i}") for i in range(3)]
            for m in mv:
                nc.vector.memset(m[:], 0.0)
            for t0, rc in cvch:
                srep = SRP.tile([128, 512], f32, tag="srep")
                nc.sync.dma_start(
                    srep[:, 0:rc * W],
                    bass.AP(tensor=sdr.tensor, offset=sdr[:].offset + t0 * W,
                            ap=[[0, 128], [1, rc * W]]))
                for g, (obase, osz) in enumerate(ogrp):
                    ps = PS.tile([64, 512], f32, tag="cvps")
                    for j in range(osz):
                        o = obase + j
                        dy, dx = o // MOFF, o % MOFF
                        pr = PR.tile([128, 4, W], bf16, tag="prod")
                        nc.vector.tensor_mul(
                            pr[:, 0:rc, :], f1t[:, t0:t0 + rc, :],
                            f2t[:, t0 + dy:t0 + dy + rc, dx:dx + W])
                        nc.tensor.matmul(
                            ps[0:osz, 0:rc * W], ones[:, j, 0:osz],
                            pr[:, 0:rc, :].rearrange("c a b -> c (a b)"),
                            start=(j == 0), stop=(j == osz - 1))
                    t1 = T.tile([64, 512], f32, tag="lk")
                    nc.vector.tensor_scalar_mul(t1[0:osz, 0:rc * W], ps[0:osz, 0:rc * W], 0.1)
                    nc.vector.tensor_tensor(t1[0:osz, 0:rc * W], ps[0:osz, 0:rc * W],
                                            t1[0:osz, 0:rc * W], op=ALU.max)
                    mt, ro = g // 2, (g % 2) * 64
                    nc.vector.tensor_mul(
                        mv[mt][ro:ro + osz, 3 + t0:3 + t0 + rc, 3:131],
                        t1[0:osz, 0:rc * W].rearrange("c (a b) -> c a b", b=W),
                        srep[0:osz, 0:rc * W].rearrange("c (a b) -> c a b", b=W))

            # ---- att = depthwise7x7(mv)+b ; av = mv*att --------------------
            av = [P.tile([128, 38, 130], bf16, tag=f"av{i}", name=f"av{i}") for i in range(3)]
            for m in av:
                nc.vector.memset(m[:], 0.0)
            for ct in range(3):
                for c in range(9):
                    tca = 3 + 4 * c          # att rows t in [3,39)
                    ps = PS.tile([128, 512], f32, tag="attps")
                    for tap in range(49):
                        u, v = tap // 7, tap % 7
                        nc.tensor.matmul(
                            ps[:, 0:512], diag[:, ct, tap, :],
                            mv[ct][:, tca + u:tca + u + 4, v:v + W],
                            start=(tap == 0), stop=(tap == 48))
                    at = T.tile([128, 512], bf16, tag="att")
                    nc.vector.tensor_scalar_add(at[:], ps[:], attb[:, ct:ct + 1])
                    nc.vector.tensor_mul(
                        av[ct][:, 1 + 4 * c:5 + 4 * c, 1:129],
                        at[:].rearrange("c (a b) -> c a b", b=W),
                        mv[ct][:, tca + 3:tca + 7, 3:131])

            # ---- conv1 3x3 289->144, BN, GELU -> x -------------------------
            xt = [P.tile([128, 36, 130], bf16, tag="xt0", name="xt0"), P.tile([16, 36, 130], bf16, tag="xt1", name="xt1")]
            for m in xt:
                nc.vector.memset(m[:], 0.0)
            x1ch = [(j0, min(4, 34 - j0)) for j0 in range(0, 34, 4)]
            for mt, mm in [(0, 128), (1, 16)]:
                for j0, rc in x1ch:
                    ps = PS.tile([128, 512], f32, tag="c1ps")
                    k = 0
                    for ct in range(3):
                        for tap in range(9):
                            ky, kx = tap // 3, tap % 3
                            nc.tensor.matmul(
                                ps[0:mm, 0:rc * W],
                                c1t[:, ct, tap, 128 * mt:128 * mt + mm],
                                av[ct][:, 1 + j0 + ky:1 + j0 + ky + rc, kx:kx + W],
                                start=(k == 0), stop=(k == 26))
                            k += 1
                    tt = T.tile([128, 512], f32, tag="bn1")
                    nc.vector.tensor_scalar(tt[0:mm, 0:rc * W], ps[0:mm, 0:rc * W],
                                            k1[0:mm, mt:mt + 1], b1[0:mm, mt:mt + 1],
                                            op0=ALU.mult, op1=ALU.add)
                    nc.scalar.activation(
                        xt[mt][0:mm, 1 + j0:1 + j0 + rc, 1:129],
                        tt[0:mm, 0:rc * W].rearrange("c (a b) -> c a b", b=W), ACT.Gelu)
            for mt, mm in [(0, 128), (1, 16)]:
                nc.vector.tensor_tensor(
                    xt[mt][0:mm],
                    xt[mt][0:mm],
                    bass.AP(tensor=xm.tensor, offset=0,
                            ap=[[36, mm], [1, 36], [0, 130]]),
                    op=ALU.mult)

            # ---- conv2 3x3 144->49, BN, GELU -> out ------------------------
            for c in range(8):
                ps = PS.tile([OUT, 512], f32, tag="c2ps")
                k = 0
                for ct, cm in [(0, 128), (1, 16)]:
                    for tap in range(9):
                        ky, kx = tap // 3, tap % 3
                        nc.tensor.matmul(
                            ps[:, 0:512],
                            c2t[0:cm, ct, tap, :],
                            xt[ct][0:cm, 1 + 4 * c + ky:5 + 4 * c + ky, kx:kx + W],
                            start=(k == 0), stop=(k == 17))
                        k += 1
                tt = T.tile([OUT, 512], f32, tag="bn2")
                nc.vector.tensor_scalar(tt[:], ps[:], k2[0:OUT, 0:1], b2[0:OUT, 0:1],
                                        op0=ALU.mult, op1=ALU.add)
                ot = T.tile([OUT, 512], f32, tag="out")
                nc.scalar.activation(ot[:], tt[:], ACT.Gelu)
                nc.sync.dma_start(outd[:, 4 * c:4 * c + 4, :],
                                  ot[:].rearrange("c (a b) -> c a b", b=W))
    nc.compile()
    return nc


def _prep(f1, f2, att_w, att_b, c1_w, bn1_g, bn1_b, bn1_m, bn1_v,
          c2_w, bn2_g, bn2_b, bn2_m, bn2_v):
    # shared weight tensors
    ones = np.zeros((128, 64, 64), BF16)
    ones[:, np.arange(64), np.arange(64)] = 1
    diag = np.zeros((128, 3, 49, 128), BF16)
    aw = att_w.reshape(NOFF, 49)
    for ct in range(3):
        n = min(128, NOFF - 128 * ct)
        for tap in range(49):
            diag[np.arange(n), ct, tap, np.arange(n)] = aw[128 * ct:128 * ct + n, tap].astype(BF16)
    attb = np.zeros((128, 3), np.float32)
    for ct in range(3):
        n = min(128, NOFF - 128 * ct)
        attb[0:n, ct] = att_b[128 * ct:128 * ct + n]
    c1t = np.zeros((128, 3, 9, MID), BF16)
    for ct in range(3):
        n = min(128, NOFF - 128 * ct)
        # c1_w [144, 289, 3, 3] -> [c_local, ct, tap, m]
        c1t[0:n, ct, :, :] = c1_w[:, 128 * ct:128 * ct + n, :, :].transpose(1, 2, 3, 0) \
            .reshape(n, 9, MID).astype(BF16)
    kv1 = (bn1_g / np.sqrt(bn1_v + 1e-5)).astype(np.float32)
    bv1 = (bn1_b - bn1_m * kv1).astype(np.float32)
    k1 = np.zeros((128, 2), np.float32)
    b1 = np.zeros((128, 2), np.float32)
    k1[:, 0], k1[0:16, 1] = kv1[0:128], kv1[128:144]
    b1[:, 0], b1[0:16, 1] = bv1[0:128], bv1[128:144]
    c2t = np.zeros((128, 2, 9, OUT), BF16)
    for ct, cm in [(0, 128), (1, 16)]:
        c2t[0:cm, ct, :, :] = c2_w[:, 128 * ct:128 * ct + cm, :, :].transpose(1, 2, 3, 0) \
            .reshape(cm, 9, OUT).astype(BF16)
    kv2 = (bn2_g / np.sqrt(bn2_v + 1e-5)).astype(np.float32)
    bv2 = (bn2_b - bn2_m * kv2).astype(np.float32)
    k2 = np.zeros((128, 1), np.float32)
    b2 = np.zeros((128, 1), np.float32)
    k2[0:OUT, 0], b2[0:OUT, 0] = kv2, bv2
    shared = dict(onescols=ones, diag_aw=diag, attb=attb, c1t=c1t, k1=k1, b1p=b1,
                  c2t=c2t, k2=k2, b2p=b2)

    in_maps = []
    for core in range(8):
        b, i = core // NBLK, core % NBLK
        r0 = BR * i
        f1s = np.zeros((128, MR, W), BF16)
        lo, hi = r0 - 5, r0 + BR + 5
        clo, chi = max(lo, 0), min(hi, H)
        f1s[:, clo - lo:chi - lo, :] = f1[b, :, clo:chi, :].astype(BF16)
        f2s = np.zeros((128, F2R, W2), BF16)
        lo2, hi2 = r0 - 13, r0 + BR + 13
        clo2, chi2 = max(lo2, 0), min(hi2, H)
        f2s[:, clo2 - lo2:chi2 - lo2, 8:8 + W] = f2[b, :, clo2:chi2, :].astype(BF16)
        xmask = np.zeros((128, 36), BF16)
        for j in range(1, 35):
            g = r0 - 2 + j
            if 0 <= g < H:
                xmask[:, j] = 1
        in_maps.append(dict(f1s=f1s, f2s=f2s, xmask=xmask, **shared))
    return in_maps


def kernel(**inputs):
    from concourse.bass_utils import run_bass_kernel_spmd
    if "nc" not in _cache:
        _cache["nc"] = _build()
    nc = _cache["nc"]
    in_maps = _prep(**inputs)
    res = run_bass_kernel_spmd(nc, in_maps, core_ids=list(range(8)))
    out = np.zeros((BS, OUT, H, W), np.float32)
    for core in range(8):
        b, i = core // NBLK, core % NBLK
        out[b, :, BR * i:BR * i + BR, :] = res.results[core]["y"]
    return out
